# revision 51
# baseline (speedup 1.0000x reference)
"""Causal self-attention (B=4, T=2048, C=1024, H=16, D=64) on 8 trn2 cores.

Sharding: data-parallel over B (4) x tensor-parallel over head-halves (2).
Core c handles batch c//2 with heads [8*(c%2), 8*(c%2)+8). Each core emits a
partial projection output [2048, 1024]; host sums the two head-half partials
per batch and adds the (bv @ Wp + bp) correction row.

Mixed-precision layout (tuned against the rel_err<2e-2 gate; see numerics
study): the logit path (x, Wq/Wk, Q^T/K^T, S) runs in bf16 — fp8 there costs
3e-2 of error through the softmax. The attention-weight/value path runs in
fp8e4 DoubleRow: exp() writes fp8 tiles directly, and V rides as a hi/lo fp8
pair (P=e4(16v), Q=e4(16v-P), ~bf16 precision) so each AV matmul contracts
two 128-row slabs per cycle-row:
  O^T[65,t] += [P|Q]^T @ [et|et]   (rhs is the same et strip, stride-0)
Row 64 of O^T is Z (ones column in P, zeros in Q). Normalize: reciprocal of
the Z row in place, SBUF->SBUF DMA broadcast of 1/Z down 64 partitions, then
one fused (O*0.25)*(1/Z) -> otc = 4y bf16; Wp is uploaded as bf16(Wp)/4 so
the bf16 projection emits y exactly. Emission is generator-quanta: attention
head PAIRS are interleaved (two softmax chains in flight per psum ring) and
QKV/proj groups are merged in as PE filler while ACT runs the exps.
"""

import os
import sys

for _p in ("/opt/trn_rl_repo", "/root/.axon_site/_ro/trn_rl_repo"):
    if os.path.isdir(_p) and _p not in sys.path:
        sys.path.insert(0, _p)

import numpy as np
import ml_dtypes
from concourse import bacc, mybir, tile
from concourse.bass_utils import run_bass_kernel_spmd

N_CORES = 8
B, T, C = 4, 2048, 1024
H, D = 16, 64          # full model heads
HG = 8                 # heads per core (head-group)
CH = HG * D            # 512, per-core qkv width
NT = T // 128          # 16 s-tiles
NJ = T // 512          # 4 t-chunks
NC_ = C // 128         # 8 contraction tiles
F32 = mybir.dt.float32
F32R = mybir.dt.float32r
BF16 = mybir.dt.bfloat16
F8 = mybir.dt.float8e4
AF = mybir.ActivationFunctionType
DR = mybir.MatmulPerfMode.DoubleRow
MUL = mybir.AluOpType.mult
SUB = mybir.AluOpType.subtract

NP_BF16 = ml_dtypes.bfloat16
NP_E4 = ml_dtypes.float8_e4m3

_CACHE = {}


def _emit(nc, tc, aps, dbg=None):
    xT, wq, wk, wv, wp, bq2, bk2, mask, yout = (
        aps["xT"], aps["wq"], aps["wk"], aps["wv"], aps["wp"],
        aps["bq2"], aps["bk2"], aps["mask"], aps["y"],
    )

    pool = tc.alloc_tile_pool(name="pool", bufs=1)
    psp = tc.alloc_tile_pool(name="ps", bufs=1, space="PSUM")

    # ---- persistent tensors ----
    kt = [pool.tile([128, T], BF16, name=f"kt{m}", tag="kt", bufs=4) for m in range(4)]
    # V tiles: per s-tile, 8 heads x (80 P | 80 Q) fp8 (only cols 0..64 of
    # each half are used; stride 80 keeps the DoubleRow LDWEIGHTS interleave
    # step 16B-aligned). P holds e4(16v) + ones column at 64; Q holds the e4
    # residual + zero column.
    vp = [pool.tile([128, 1280], F8, name=f"vp{i}", tag="vp", bufs=NT)
          for i in range(NT)]
    tri = pool.tile([128, 128], F8, name="tri", tag="tri", bufs=1)
    bqs = pool.tile([128, 4], F32, name="bqs", tag="bias", bufs=2)
    bks = pool.tile([128, 4], F32, name="bks", tag="bias", bufs=2)
    onesf = pool.tile([128, 8], F32, name="onesf", tag="ones", bufs=2)
    zerof = pool.tile([128, 8], F32, name="zerof", tag="ones", bufs=2)
    nb1 = pool.tile([128, 1], F32, name="nb1", tag="nb", bufs=1)
    onesA = pool.tile([128, 64], F32, name="onesA", tag="onesA", bufs=2)
    ones64 = pool.tile([128, 64], F32R, name="ones64", tag="onesA", bufs=2)

    # weights as single [128, 4096] tiles (one merged DMA each; the shared
    # HWDGE device costs ~625ns per DMA instruction, so fewer+bigger wins)
    wqb = pool.tile([128, 4096], BF16, name="wqb", tag="w", bufs=4)
    wkb = pool.tile([128, 4096], BF16, name="wkb", tag="w", bufs=4)
    wvb = pool.tile([128, 4096], BF16, name="wvb", tag="w", bufs=4)
    wqs = [wqb[:, 512 * ci:512 * ci + 512] for ci in range(NC_)]
    wks = [wkb[:, 512 * ci:512 * ci + 512] for ci in range(NC_)]
    wvs = [wvb[:, 512 * ci:512 * ci + 512] for ci in range(NC_)]
    xbig = [pool.tile([128, 4096], BF16, name=f"xb{j}", tag="xt", bufs=4)
            for j in range(NJ)]

    def dma_w(eng, dst, src, lo=0, hi=8):
        eng.dma_start(
            dst[:, 512 * lo:512 * hi].rearrange("p (ci n) -> p ci n", ci=hi - lo),
            src[:, :].rearrange("(ci p) n -> p ci n", ci=8)[:, lo:hi],
        )

    def dma_x(eng, j, lo=0, hi=8):
        eng.dma_start(
            xbig[j][:, 512 * lo:512 * hi].rearrange("p (ci t) -> p ci t",
                                                    ci=hi - lo),
            xT[:, :].rearrange("(ci p) t -> p ci t", ci=8)
            [:, lo:hi, 512 * j:512 * j + 512],
        )

    # first-use tensors split across both HWDGE queues so the first QKV
    # groups can start ~2x sooner
    dma_w(nc.sync, wqb, wq, 0, 2)
    dma_x(nc.scalar, 0, 0, 2)
    dma_w(nc.sync, wqb, wq, 2, 4)
    dma_x(nc.scalar, 0, 2, 4)
    dma_w(nc.sync, wqb, wq, 4, 6)
    dma_x(nc.scalar, 0, 4, 6)
    dma_w(nc.sync, wqb, wq, 6, 8)
    dma_x(nc.scalar, 0, 6, 8)
    dma_w(nc.sync, wkb, wk, 0, 4)
    dma_w(nc.sync, wkb, wk, 4, 8)
    nc.scalar.dma_start(bqs[:], bq2[:])
    nc.scalar.dma_start(bks[:], bk2[:])
    dma_w(nc.scalar, wvb, wv)
    nc.scalar.dma_start(tri[:], mask[:])
    xt0 = [xbig[0][:, 512 * ci:512 * ci + 512] for ci in range(NC_)]
    # walrus rejects memsets with exotic value/dtype combos; stick to fp32
    # 0.0/1.0 memsets and derive everything else on DVE
    nc.gpsimd.memset(onesf[:], 1.0)
    nc.gpsimd.memset(zerof[:], 0.0)
    nc.gpsimd.memset(nb1[:], 1.0)
    nc.vector.tensor_scalar_mul(nb1[:], nb1[:], -1.0)
    nc.gpsimd.memset(onesA[:], 1.0)
    nc.vector.tensor_copy(ones64[:], onesA[:])
    for i in range(NT):
        vpr = vp[i][:, 0:1280].rearrange("p (h two e) -> p h two e", two=2, e=80)
        nc.vector.tensor_copy(vpr[:, :, 0, 64:65], onesf[:].unsqueeze(2))
        nc.vector.tensor_copy(vpr[:, :, 1, 64:65], zerof[:].unsqueeze(2))

    qtc = [[None] * NJ for _ in range(4)]   # per-chunk Q^T tiles (bf16)
    otc = [[None] * NJ for _ in range(4)]   # per-chunk otc tiles (bf16, =4y)
    wps = [[None, None] for _ in range(4)]  # wp [128,512] halves, loaded late

    def gen_qkv(j):
        """Yields after each PE group: 1 dma quantum + 12 matmul quanta."""
        if j == 0:
            xts = xt0
        else:
            dma_x(nc.sync, j)
            xts = [xbig[j][:, 512 * ci:512 * ci + 512] for ci in range(NC_)]
            yield
        for wsrc, bias_t, dst, nm in ((wqs, bqs, qtc, "qt"), (wks, bks, None, "kt")):
            for m in range(4):
                ps = psp.tile([128, 512], F32, name=f"{nm}ps{j}_{m}", tag="qk", bufs=2)
                for ci in range(NC_):
                    nc.tensor.matmul(
                        ps[:], wsrc[ci][:, 128 * m:128 * m + 128], xts[ci][:],
                        start=(ci == 0), stop=(ci == NC_ - 1),
                    )
                if dst is None:
                    out_ap = kt[m][:, 512 * j:512 * j + 512]
                else:
                    t_ = pool.tile([128, 512], BF16, name=f"qt{m}_{j}", tag="qtc",
                                   bufs=8)
                    dst[m][j] = t_
                    out_ap = t_[:]
                nc.vector.tensor_scalar_add(out_ap, ps[:], bias_t[:, m:m + 1])
                if dbg is not None and j == 0 and m == 0:
                    if dst is None:
                        nc.scalar.dma_start(dbg["kt0"][:, :],
                                            kt[0][:, 0:512])
                    else:
                        nc.scalar.dma_start(dbg["qt0"][:, :], t_[:])
                yield
        for u in range(4):
            i = 4 * j + u
            ps = psp.tile([128, 512], F32, name=f"vps{i}", tag="qk", bufs=2)
            for ci in range(NC_):
                nc.tensor.matmul(
                    ps[:], xts[ci][:, 128 * u:128 * u + 128], wvs[ci][:],
                    start=(ci == 0), stop=(ci == NC_ - 1),
                )
            vpr = vp[i][:, 0:1280].rearrange("p (h two e) -> p h two e",
                                             two=2, e=80)
            src = ps[:].rearrange("p (h e) -> p h e", e=64)
            # P = e4(16 v); Q = e4(16 v - P)
            nc.vector.tensor_scalar_mul(vpr[:, :, 0, 0:64], src, 16.0)
            nc.vector.scalar_tensor_tensor(
                vpr[:, :, 1, 0:64], in0=src, scalar=16.0,
                in1=vpr[:, :, 0, 0:64], op0=MUL, op1=SUB,
            )
            if dbg is not None and i == 0:
                nc.scalar.dma_start(dbg["vp0"][:, :], vp[0][:])
            yield

    def gen_attn(j, heads=(1, 0, 3, 2, 5, 4, 7, 6)):
        n_i = 4 * j + 4

        def tile_layout(p):
            # pairs of s-tiles per [128,1024] PSUM slot; diagonal tiles are
            # narrowed to the causally valid t-range [128r, 512).
            # entries: (i, slot_col, valid_t0, width, diag_block_col)
            i0, i1 = 2 * p, 2 * p + 1
            r0_, r1_ = i0 - 4 * j, i1 - 4 * j
            if r1_ < 0:
                return [(i0, 0, 0, 512, None), (i1, 512, 0, 512, None)], 1024
            if r0_ == 0:
                return [(i0, 0, 0, 512, 0), (i1, 512, 128, 384, 512)], 896
            return [(i0, 0, 256, 256, 0), (i1, 256, 384, 128, 256)], 384

        # odd heads first: their normalize chain ends in a partition-shifting
        # SBUF->SBUF DMA, so keep an even (cheap-chain) head last
        def head_gen(h):
            mt = h // 2
            off = 64 * (h % 2)
            ops = psp.tile([65, 512], F32, name=f"ops{h}_{j}", tag="o", bufs=2)
            qsrc = qtc[mt][j][off:off + 64, :]
            first_av = True

            def emit_av(layout, et, p):
                nonlocal first_av
                for (i, scol, t0, w, dcol) in layout:
                    if dcol is not None:
                        blk = et[:, dcol:dcol + 128]
                        nc.gpsimd.tensor_mul(blk, blk, tri[:])
                    vp_lhsT = vp[i][:, 160 * h:160 * h + 160].rearrange(
                        "p (two e) -> p two e", two=2)[:, :, 0:65]
                    for c0 in range(0, w, 256):
                        wc = min(256, w - c0)
                        rhs = et[:, scol + c0:scol + c0 + wc].unsqueeze(1) \
                            .broadcast_to([128, 2, wc])
                        nc.tensor.matmul(
                            ops[:, t0 + c0:t0 + c0 + wc], vp_lhsT, rhs,
                            start=first_av, stop=(i == n_i - 1 and c0 + wc == w),
                            perf_mode=DR,
                        )
                        first_av = False
                if dbg is not None and h == 0 and j == 0 and p == 0:
                    nc.scalar.dma_start(dbg["et00"][:, :], et[:])

            pending = None   # software pipeline: AV(p) emitted after S(p+1)
            for p in range(n_i // 2):
                layout, exp_hi = tile_layout(p)
                sp = psp.tile([128, 1024], F32, name=f"sp{h}_{j}_{p}", tag="sp",
                              bufs=2)
                for (i, scol, t0, w, _) in layout:
                    nc.tensor.matmul(
                        sp[:, scol:scol + w],
                        kt[mt][off:off + 64, 128 * i:128 * i + 128],
                        qsrc[:, t0:t0 + w],
                        start=True, stop=True,
                    )
                et = pool.tile([128, 1024], F8, name=f"et{h}_{j}_{p}", tag="et",
                               bufs=6)
                nc.scalar.activation(et[:, 0:exp_hi], sp[:, 0:exp_hi], AF.Exp,
                                     scale=0.125, bias=nb1[:])
                if pending is not None:
                    emit_av(*pending)
                pending = (layout, et, p)
                yield
            emit_av(*pending)
            # normalize: rows 0..63 unnormalized O^T (=16 y Z), row 64 = Z
            # 1/Z in place at partition 64 (fp32r), PE-broadcast to the 64
            # O^T partitions, then one fused (O * 0.25) * (1/Z) -> otc = 4y
            zr = pool.tile([65, 512], F32R, name=f"zr{h}_{j}", tag="zr", bufs=2)
            with nc.allow_low_precision(reason="fp32r rounding of softmax denom"):
                nc.vector.reciprocal(zr[64:65, :], ops[64:65, :])
            rbs = pool.tile([64, 512], F32R, name=f"rbs{h}_{j}", tag="rbs", bufs=2)
            if j == 3 and h in (6, 7):
                # tail: PE is idle and the DMA round-trip would sit on the
                # critical path into proj3 -- broadcast via PE instead
                rbp = psp.tile([64, 512], F32, name=f"rbp{h}", tag="qk", bufs=2)
                nc.tensor.matmul(rbp[:], ones64[64:65, :], zr[64:65, :],
                                 start=True, stop=True)
                nc.vector.tensor_copy(rbs[:], rbp[:])
            else:
                with nc.allow_non_contiguous_dma(reason="1/Z partition broadcast"):
                    nc.sync.dma_start(
                        rbs[:],
                        zr[64:65, :].unsqueeze(1).broadcast_to([1, 64, 512]))
            if dbg is not None and h == 0 and j == 0:
                opc = pool.tile([65, 512], F32, name="dbgopc", tag="dbgo", bufs=1)
                nc.vector.tensor_copy(opc[:], ops[:])
                nc.scalar.dma_start(dbg["ops00"][:, :], opc[:])
                nc.scalar.dma_start(dbg["rbs00"][:, :], rbs[:].bitcast(F32))
            if otc[mt][j] is None:
                otc[mt][j] = pool.tile([128, 512], BF16, name=f"ot{mt}_{j}",
                                       tag="otc", bufs=16)
            if h % 2 == 0:
                nc.vector.scalar_tensor_tensor(
                    otc[mt][j][0:64, :], in0=ops[0:64, :], scalar=0.25,
                    in1=rbs[:], op0=MUL, op1=MUL)
                if dbg is not None and h == 0 and j == 0:
                    nc.scalar.dma_start(dbg["otc00"][:, :], otc[0][0][0:64, :])
            else:
                st = pool.tile([64, 512], BF16, name=f"st{h}_{j}", tag="st", bufs=1)
                nc.vector.scalar_tensor_tensor(
                    st[:], in0=ops[0:64, :], scalar=0.25,
                    in1=rbs[:], op0=MUL, op1=MUL)
                nc.sync.dma_start(otc[mt][j][64:128, :], st[:])
            yield

        for ha, hb in zip(heads[0::2], heads[1::2]):
            alive = [head_gen(ha), head_gen(hb)]
            while alive:
                for g in list(alive):
                    try:
                        next(g)
                    except StopIteration:
                        alive.remove(g)
                    else:
                        yield

    def gen_wp_loads():
        wpb = pool.tile([128, 4096], BF16, name="wpb", tag="w", bufs=4)
        nc.sync.dma_start(
            wpb[:].rearrange("p (m c) -> p m c", m=4),
            wp[:, :].rearrange("(m p) c -> p m c", m=4),
        )
        for m in range(4):
            for n in range(2):
                wps[m][n] = wpb[:, 1024 * m + 512 * n:1024 * m + 512 * n + 512]
        yield

    def gen_proj(j, overlap=False):
        start_u = 0
        if overlap and j == 3:
            # open two psum groups with m=0..2 while the last attention pair
            # is still in flight; m=3 closes them once otc[3][3] exists
            t = 12
            yo = pool.tile([128, 1024], F32, name=f"yo{t}", tag="yo", bufs=2)
            pss = []
            for n in range(2):
                ps = psp.tile([128, 512], F32, name=f"yps{t}_{n}", tag="qk",
                              bufs=2)
                for m in range(3):
                    nc.tensor.matmul(
                        ps[:], otc[m][j][:, 0:128], wps[m][n],
                        start=(m == 0), stop=False,
                    )
                pss.append(ps)
                yield
            for n in range(2):
                nc.tensor.matmul(
                    pss[n][:], otc[3][j][:, 0:128], wps[3][n],
                    start=False, stop=True,
                )
                nc.scalar.copy(yo[:, 512 * n:512 * n + 512], pss[n][:])
                yield
            nc.sync.dma_start(yout[128 * t:128 * t + 128, :], yo[:])
            start_u = 1
        for u in range(start_u, 4):
            t = 4 * j + u
            yo = pool.tile([128, 1024], F32, name=f"yo{t}", tag="yo", bufs=2)
            for n in range(2):
                ps = psp.tile([128, 512], F32, name=f"yps{t}_{n}", tag="qk", bufs=2)
                for m in range(4):
                    nc.tensor.matmul(
                        ps[:], otc[m][j][:, 128 * u:128 * u + 128], wps[m][n],
                        start=(m == 0), stop=(m == 3),
                    )
                if j == 3:
                    nc.scalar.copy(yo[:, 512 * n:512 * n + 512], ps[:])
                else:
                    nc.vector.tensor_copy(yo[:, 512 * n:512 * n + 512], ps[:])
                yield
            nc.sync.dma_start(yout[128 * t:128 * t + 128, :], yo[:])

    def chain(*gens):
        for g in gens:
            yield from g

    def merge(main, filler, ratio):
        """Pull `ratio` quanta from main, then 1 from filler, until both dry."""
        main_live = filler_live = True
        while main_live or filler_live:
            for _ in range(ratio):
                if main_live:
                    try:
                        next(main)
                    except StopIteration:
                        main_live = False
            if filler_live:
                try:
                    next(filler)
                except StopIteration:
                    filler_live = False

    def drain(g):
        for _ in g:
            pass

    drain(gen_qkv(0))
    merge(gen_attn(0), gen_qkv(1), 2)
    merge(gen_attn(1), gen_qkv(2), 3)
    merge(gen_attn(2), chain(gen_qkv(3), gen_wp_loads(), gen_proj(0)), 3)
    merge(gen_attn(3),
          chain(gen_proj(1), gen_proj(2), gen_proj(3, overlap=True)), 4)

    for m in range(4):
        qtc[m] = [None] * NJ
        otc[m] = [None] * NJ
    pool.release()
    psp.release()


def build(passes=1, dbg=False):
    key = ("nc", passes, dbg)
    if key in _CACHE:
        return _CACHE[key]
    nc = bacc.Bacc("TRN2", target_bir_lowering=False, debug=False,
                   num_devices=N_CORES)
    aps = {
        "xT": nc.dram_tensor("xT", [C, T], BF16, kind="ExternalInput").ap(),
        "wq": nc.dram_tensor("wq", [C, CH], BF16, kind="ExternalInput").ap(),
        "wk": nc.dram_tensor("wk", [C, CH], BF16, kind="ExternalInput").ap(),
        "wv": nc.dram_tensor("wv", [C, CH], BF16, kind="ExternalInput").ap(),
        "wp": nc.dram_tensor("wp", [CH, C], BF16, kind="ExternalInput").ap(),
        "bq2": nc.dram_tensor("bq2", [128, 4], F32, kind="ExternalInput").ap(),
        "bk2": nc.dram_tensor("bk2", [128, 4], F32, kind="ExternalInput").ap(),
        "mask": nc.dram_tensor("mask", [128, 128], F8, kind="ExternalInput").ap(),
        "y": nc.dram_tensor("y", [T, C], F32, kind="ExternalOutput").ap(),
    }
    dbg_aps = None
    if dbg:
        dbg_aps = {
            "qt0": nc.dram_tensor("dqt0", [128, 512], BF16,
                                  kind="ExternalOutput").ap(),
            "kt0": nc.dram_tensor("dkt0", [128, 512], BF16,
                                  kind="ExternalOutput").ap(),
            "vp0": nc.dram_tensor("dvp0", [128, 1280], F8,
                                  kind="ExternalOutput").ap(),
            "et00": nc.dram_tensor("det00", [128, 1024], F8,
                                   kind="ExternalOutput").ap(),
            "ops00": nc.dram_tensor("dops00", [65, 512], F32,
                                    kind="ExternalOutput").ap(),
            "rbs00": nc.dram_tensor("drbs00", [64, 512], F32,
                                    kind="ExternalOutput").ap(),
            "otc00": nc.dram_tensor("dotc00", [64, 512], BF16,
                                    kind="ExternalOutput").ap(),
        }
    with tile.TileContext(nc) as tc:
        for _ in range(passes):
            _emit(nc, tc, aps, dbg=dbg_aps)
    nc.compile()
    _CACHE[key] = nc
    return nc


def make_in_maps(x, Wq, bq, Wk, bk, Wv, bv, Wp, bp):
    # lower-triangle 0/1 mask (valid where s <= t) for diagonal blocks
    s_idx = np.arange(128)[:, None]
    t_idx = np.arange(128)[None, :]
    mask = (s_idx <= t_idx).astype(NP_E4)
    in_maps = []
    for c in range(N_CORES):
        b, g = c // 2, c % 2
        cols = slice(CH * g, CH * g + CH)
        in_maps.append({
            "xT": np.ascontiguousarray(x[b].T).astype(NP_BF16),
            "wq": np.ascontiguousarray(Wq[:, cols]).astype(NP_BF16),
            "wk": np.ascontiguousarray(Wk[:, cols]).astype(NP_BF16),
            "wv": np.ascontiguousarray(Wv[:, cols]).astype(NP_BF16),
            "wp": np.ascontiguousarray(Wp[cols, :] * 0.25).astype(NP_BF16),
            "bq2": np.ascontiguousarray(bq[cols].reshape(4, 128).T),
            "bk2": np.ascontiguousarray(bk[cols].reshape(4, 128).T),
            "mask": mask,
        })
    return in_maps


def kernel(x, Wq, bq, Wk, bk, Wv, bv, Wp, bp):
    # host-side prep is pure numpy; convert in case jax arrays are passed
    x, Wq, bq, Wk, bk, Wv, bv, Wp, bp = (
        np.asarray(a, dtype=np.float32)
        for a in (x, Wq, bq, Wk, bk, Wv, bv, Wp, bp)
    )
    nc = build()
    in_maps = make_in_maps(x, Wq, bq, Wk, bk, Wv, bv, Wp, bp)
    # the axon-proxied device occasionally reports a transient unrecoverable
    # exec state that clears on a fresh attempt; retry rather than fail
    last_err = None
    for _attempt in range(3):
        try:
            res = run_bass_kernel_spmd(nc, in_maps, core_ids=list(range(N_CORES)))
            break
        except Exception as e:  # noqa: BLE001
            last_err = e
            import time as _time
            _time.sleep(5)
    else:
        raise last_err
    corr = (bv @ Wp + bp).astype(np.float32)
    out = np.empty((B, T, C), dtype=np.float32)
    for b in range(B):
        out[b] = res.results[2 * b]["y"] + res.results[2 * b + 1]["y"] + corr
    return out


# revision 54
# speedup vs baseline: 1.0016x; 1.0016x over previous
"""Causal self-attention (B=4, T=2048, C=1024, H=16, D=64) on 8 trn2 cores.

Sharding: data-parallel over B (4) x tensor-parallel over head-halves (2).
Core c handles batch c//2 with heads [8*(c%2), 8*(c%2)+8). Each core emits a
partial projection output [2048, 1024]; host sums the two head-half partials
per batch and adds the (bv @ Wp + bp) correction row.

Mixed-precision layout (tuned against the rel_err<2e-2 gate; see numerics
study): the logit path (x, Wq/Wk, Q^T/K^T, S) runs in bf16 — fp8 there costs
3e-2 of error through the softmax. The attention-weight/value path runs in
fp8e4 DoubleRow: exp() writes fp8 tiles directly, and V rides as a hi/lo fp8
pair (P=e4(16v), Q=e4(16v-P), ~bf16 precision) so each AV matmul contracts
two 128-row slabs per cycle-row:
  O^T[65,t] += [P|Q]^T @ [et|et]   (rhs is the same et strip, stride-0)
Row 64 of O^T is Z (ones column in P, zeros in Q). Normalize: reciprocal of
the Z row in place, SBUF->SBUF DMA broadcast of 1/Z down 64 partitions, then
one fused (O*0.25)*(1/Z) -> otc = 4y bf16; Wp is uploaded as bf16(Wp)/4 so
the bf16 projection emits y exactly. Emission is generator-quanta: attention
head PAIRS are interleaved (two softmax chains in flight per psum ring) and
QKV/proj groups are merged in as PE filler while ACT runs the exps.
"""

import os
import sys

for _p in ("/opt/trn_rl_repo", "/root/.axon_site/_ro/trn_rl_repo"):
    if os.path.isdir(_p) and _p not in sys.path:
        sys.path.insert(0, _p)

import numpy as np
import ml_dtypes
from concourse import bacc, mybir, tile
from concourse.bass_utils import run_bass_kernel_spmd

N_CORES = 8
B, T, C = 4, 2048, 1024
H, D = 16, 64          # full model heads
HG = 8                 # heads per core (head-group)
CH = HG * D            # 512, per-core qkv width
NT = T // 128          # 16 s-tiles
NJ = T // 512          # 4 t-chunks
NC_ = C // 128         # 8 contraction tiles
F32 = mybir.dt.float32
F32R = mybir.dt.float32r
BF16 = mybir.dt.bfloat16
F8 = mybir.dt.float8e4
AF = mybir.ActivationFunctionType
DR = mybir.MatmulPerfMode.DoubleRow
MUL = mybir.AluOpType.mult
SUB = mybir.AluOpType.subtract

NP_BF16 = ml_dtypes.bfloat16
NP_E4 = ml_dtypes.float8_e4m3

_CACHE = {}


def _emit(nc, tc, aps, dbg=None):
    xT, wq, wk, wv, wp, bq2, bk2, mask, yout = (
        aps["xT"], aps["wq"], aps["wk"], aps["wv"], aps["wp"],
        aps["bq2"], aps["bk2"], aps["mask"], aps["y"],
    )

    pool = tc.alloc_tile_pool(name="pool", bufs=1)
    psp = tc.alloc_tile_pool(name="ps", bufs=1, space="PSUM")

    # ---- persistent tensors ----
    kt = [pool.tile([128, T], BF16, name=f"kt{m}", tag="kt", bufs=4) for m in range(4)]
    # V tiles: per s-tile, 8 heads x (80 P | 80 Q) fp8 (only cols 0..64 of
    # each half are used; stride 80 keeps the DoubleRow LDWEIGHTS interleave
    # step 16B-aligned). P holds e4(16v) + ones column at 64; Q holds the e4
    # residual + zero column.
    vp = [pool.tile([128, 1280], F8, name=f"vp{i}", tag="vp", bufs=NT)
          for i in range(NT)]
    tri = pool.tile([128, 128], F8, name="tri", tag="tri", bufs=1)
    bqs = pool.tile([128, 4], F32, name="bqs", tag="bias", bufs=2)
    bks = pool.tile([128, 4], F32, name="bks", tag="bias", bufs=2)
    onesf = pool.tile([128, 8], F32, name="onesf", tag="ones", bufs=2)
    zerof = pool.tile([128, 8], F32, name="zerof", tag="ones", bufs=2)
    nb1 = pool.tile([128, 1], F32, name="nb1", tag="nb", bufs=1)
    onesA = pool.tile([128, 64], F32, name="onesA", tag="onesA", bufs=2)
    ones64 = pool.tile([128, 64], F32R, name="ones64", tag="onesA", bufs=2)

    # weights as single [128, 4096] tiles (one merged DMA each; the shared
    # HWDGE device costs ~625ns per DMA instruction, so fewer+bigger wins)
    wqb = pool.tile([128, 4096], BF16, name="wqb", tag="w", bufs=4)
    wkb = pool.tile([128, 4096], BF16, name="wkb", tag="w", bufs=4)
    wvb = pool.tile([128, 4096], BF16, name="wvb", tag="w", bufs=4)
    wqs = [wqb[:, 512 * ci:512 * ci + 512] for ci in range(NC_)]
    wks = [wkb[:, 512 * ci:512 * ci + 512] for ci in range(NC_)]
    wvs = [wvb[:, 512 * ci:512 * ci + 512] for ci in range(NC_)]
    xbig = [pool.tile([128, 4096], BF16, name=f"xb{j}", tag="xt", bufs=4)
            for j in range(NJ)]

    def dma_w(eng, dst, src, lo=0, hi=8):
        eng.dma_start(
            dst[:, 512 * lo:512 * hi].rearrange("p (ci n) -> p ci n", ci=hi - lo),
            src[:, :].rearrange("(ci p) n -> p ci n", ci=8)[:, lo:hi],
        )

    def dma_x(eng, j, lo=0, hi=8):
        eng.dma_start(
            xbig[j][:, 512 * lo:512 * hi].rearrange("p (ci t) -> p ci t",
                                                    ci=hi - lo),
            xT[:, :].rearrange("(ci p) t -> p ci t", ci=8)
            [:, lo:hi, 512 * j:512 * j + 512],
        )

    # first-use tensors split across both HWDGE queues so the first QKV
    # groups can start ~2x sooner
    dma_w(nc.sync, wqb, wq, 0, 2)
    dma_x(nc.scalar, 0, 0, 2)
    dma_w(nc.sync, wqb, wq, 2, 4)
    dma_x(nc.scalar, 0, 2, 4)
    dma_w(nc.sync, wqb, wq, 4, 6)
    dma_x(nc.scalar, 0, 4, 6)
    dma_w(nc.sync, wqb, wq, 6, 8)
    dma_x(nc.scalar, 0, 6, 8)
    dma_w(nc.sync, wkb, wk, 0, 4)
    dma_w(nc.sync, wkb, wk, 4, 8)
    nc.scalar.dma_start(bqs[:], bq2[:])
    nc.scalar.dma_start(bks[:], bk2[:])
    dma_w(nc.scalar, wvb, wv)
    nc.scalar.dma_start(tri[:], mask[:])
    xt0 = [xbig[0][:, 512 * ci:512 * ci + 512] for ci in range(NC_)]
    # walrus rejects memsets with exotic value/dtype combos; stick to fp32
    # 0.0/1.0 memsets and derive everything else on DVE
    nc.gpsimd.memset(onesf[:], 1.0)
    nc.gpsimd.memset(zerof[:], 0.0)
    nc.gpsimd.memset(nb1[:], 1.0)
    nc.vector.tensor_scalar_mul(nb1[:], nb1[:], -1.0)
    nc.gpsimd.memset(onesA[:], 1.0)
    nc.vector.tensor_copy(ones64[:], onesA[:])
    for i in range(NT):
        vpr = vp[i][:, 0:1280].rearrange("p (h two e) -> p h two e", two=2, e=80)
        nc.vector.tensor_copy(vpr[:, :, 0, 64:65], onesf[:].unsqueeze(2))
        nc.vector.tensor_copy(vpr[:, :, 1, 64:65], zerof[:].unsqueeze(2))

    qtc = [[None] * NJ for _ in range(4)]   # per-chunk Q^T tiles (bf16)
    otc = [[None] * NJ for _ in range(4)]   # per-chunk otc tiles (bf16, =4y)
    wps = [[None, None] for _ in range(4)]  # wp [128,512] halves, loaded late

    def gen_qkv(j):
        """Yields after each PE group: 1 dma quantum + 12 matmul quanta."""
        if j == 0:
            xts = xt0
        else:
            dma_x(nc.sync, j)
            xts = [xbig[j][:, 512 * ci:512 * ci + 512] for ci in range(NC_)]
            yield
        for wsrc, bias_t, dst, nm in ((wqs, bqs, qtc, "qt"), (wks, bks, None, "kt")):
            for m in range(4):
                ps = psp.tile([128, 512], F32, name=f"{nm}ps{j}_{m}", tag="qk", bufs=2)
                for ci in range(NC_):
                    nc.tensor.matmul(
                        ps[:], wsrc[ci][:, 128 * m:128 * m + 128], xts[ci][:],
                        start=(ci == 0), stop=(ci == NC_ - 1),
                    )
                if dst is None:
                    out_ap = kt[m][:, 512 * j:512 * j + 512]
                else:
                    t_ = pool.tile([128, 512], BF16, name=f"qt{m}_{j}", tag="qtc",
                                   bufs=8)
                    dst[m][j] = t_
                    out_ap = t_[:]
                nc.vector.tensor_scalar_add(out_ap, ps[:], bias_t[:, m:m + 1])
                if dbg is not None and j == 0 and m == 0:
                    if dst is None:
                        nc.scalar.dma_start(dbg["kt0"][:, :],
                                            kt[0][:, 0:512])
                    else:
                        nc.scalar.dma_start(dbg["qt0"][:, :], t_[:])
                yield
        for u in range(4):
            i = 4 * j + u
            ps = psp.tile([128, 512], F32, name=f"vps{i}", tag="qk", bufs=2)
            for ci in range(NC_):
                nc.tensor.matmul(
                    ps[:], xts[ci][:, 128 * u:128 * u + 128], wvs[ci][:],
                    start=(ci == 0), stop=(ci == NC_ - 1),
                )
            vpr = vp[i][:, 0:1280].rearrange("p (h two e) -> p h two e",
                                             two=2, e=80)
            src = ps[:].rearrange("p (h e) -> p h e", e=64)
            # P = e4(16 v); Q = e4(16 v - P)
            nc.vector.tensor_scalar_mul(vpr[:, :, 0, 0:64], src, 16.0)
            nc.vector.scalar_tensor_tensor(
                vpr[:, :, 1, 0:64], in0=src, scalar=16.0,
                in1=vpr[:, :, 0, 0:64], op0=MUL, op1=SUB,
            )
            if dbg is not None and i == 0:
                nc.scalar.dma_start(dbg["vp0"][:, :], vp[0][:])
            yield

    def gen_attn(j, heads=(1, 0, 3, 2, 5, 4, 7, 6)):
        n_i = 4 * j + 4

        def tile_layout(p):
            # pairs of s-tiles per [128,1024] PSUM slot; diagonal tiles are
            # narrowed to the causally valid t-range [128r, 512).
            # entries: (i, slot_col, valid_t0, width, diag_block_col)
            i0, i1 = 2 * p, 2 * p + 1
            r0_, r1_ = i0 - 4 * j, i1 - 4 * j
            if r1_ < 0:
                return [(i0, 0, 0, 512, None), (i1, 512, 0, 512, None)], 1024
            if r0_ == 0:
                return [(i0, 0, 0, 512, 0), (i1, 512, 128, 384, 512)], 896
            return [(i0, 0, 256, 256, 0), (i1, 256, 384, 128, 256)], 384

        # odd heads first: their normalize chain ends in a partition-shifting
        # SBUF->SBUF DMA, so keep an even (cheap-chain) head last
        def head_gen(h):
            mt = h // 2
            off = 64 * (h % 2)
            ops = psp.tile([65, 512], F32, name=f"ops{h}_{j}", tag="o", bufs=2)
            qsrc = qtc[mt][j][off:off + 64, :]
            first_av = True

            def emit_av(layout, et, p):
                nonlocal first_av
                for (i, scol, t0, w, dcol) in layout:
                    if dcol is not None:
                        blk = et[:, dcol:dcol + 128]
                        nc.gpsimd.tensor_mul(blk, blk, tri[:])
                    vp_lhsT = vp[i][:, 160 * h:160 * h + 160].rearrange(
                        "p (two e) -> p two e", two=2)[:, :, 0:65]
                    for c0 in range(0, w, 256):
                        wc = min(256, w - c0)
                        rhs = et[:, scol + c0:scol + c0 + wc].unsqueeze(1) \
                            .broadcast_to([128, 2, wc])
                        nc.tensor.matmul(
                            ops[:, t0 + c0:t0 + c0 + wc], vp_lhsT, rhs,
                            start=first_av, stop=(i == n_i - 1 and c0 + wc == w),
                            perf_mode=DR,
                        )
                        first_av = False
                if dbg is not None and h == 0 and j == 0 and p == 0:
                    nc.scalar.dma_start(dbg["et00"][:, :], et[:])

            pending = None   # software pipeline: AV(p) emitted after S(p+1)
            for p in range(n_i // 2):
                layout, exp_hi = tile_layout(p)
                sp = psp.tile([128, 1024], F32, name=f"sp{h}_{j}_{p}", tag="sp",
                              bufs=2)
                for (i, scol, t0, w, _) in layout:
                    nc.tensor.matmul(
                        sp[:, scol:scol + w],
                        kt[mt][off:off + 64, 128 * i:128 * i + 128],
                        qsrc[:, t0:t0 + w],
                        start=True, stop=True,
                    )
                et = pool.tile([128, 1024], F8, name=f"et{h}_{j}_{p}", tag="et",
                               bufs=6)
                nc.scalar.activation(et[:, 0:exp_hi], sp[:, 0:exp_hi], AF.Exp,
                                     scale=0.125, bias=nb1[:])
                if pending is not None:
                    emit_av(*pending)
                pending = (layout, et, p)
                yield
            emit_av(*pending)
            # normalize: rows 0..63 unnormalized O^T (=16 y Z), row 64 = Z
            # 1/Z in place at partition 64 (fp32r), PE-broadcast to the 64
            # O^T partitions, then one fused (O * 0.25) * (1/Z) -> otc = 4y
            zr = pool.tile([65, 512], F32R, name=f"zr{h}_{j}", tag="zr", bufs=2)
            with nc.allow_low_precision(reason="fp32r rounding of softmax denom"):
                nc.vector.reciprocal(zr[64:65, :], ops[64:65, :])
            rbs = pool.tile([64, 512], F32R, name=f"rbs{h}_{j}", tag="rbs", bufs=2)
            if j == 3 and h in (6, 7):
                # tail: PE is idle and the DMA round-trip would sit on the
                # critical path into proj3 -- broadcast via PE instead
                rbp = psp.tile([64, 512], F32, name=f"rbp{h}", tag="qk", bufs=2)
                nc.tensor.matmul(rbp[:], ones64[64:65, :], zr[64:65, :],
                                 start=True, stop=True)
                nc.vector.tensor_copy(rbs[:], rbp[:])
            else:
                with nc.allow_non_contiguous_dma(reason="1/Z partition broadcast"):
                    nc.sync.dma_start(
                        rbs[:],
                        zr[64:65, :].unsqueeze(1).broadcast_to([1, 64, 512]))
            if dbg is not None and h == 0 and j == 0:
                opc = pool.tile([65, 512], F32, name="dbgopc", tag="dbgo", bufs=1)
                nc.vector.tensor_copy(opc[:], ops[:])
                nc.scalar.dma_start(dbg["ops00"][:, :], opc[:])
                nc.scalar.dma_start(dbg["rbs00"][:, :], rbs[:].bitcast(F32))
            if otc[mt][j] is None:
                otc[mt][j] = pool.tile([128, 512], BF16, name=f"ot{mt}_{j}",
                                       tag="otc", bufs=16)
            if h % 2 == 0:
                nc.vector.scalar_tensor_tensor(
                    otc[mt][j][0:64, :], in0=ops[0:64, :], scalar=0.25,
                    in1=rbs[:], op0=MUL, op1=MUL)
                if dbg is not None and h == 0 and j == 0:
                    nc.scalar.dma_start(dbg["otc00"][:, :], otc[0][0][0:64, :])
            else:
                st = pool.tile([64, 512], BF16, name=f"st{h}_{j}", tag="st", bufs=1)
                nc.vector.scalar_tensor_tensor(
                    st[:], in0=ops[0:64, :], scalar=0.25,
                    in1=rbs[:], op0=MUL, op1=MUL)
                nc.sync.dma_start(otc[mt][j][64:128, :], st[:])
            yield

        for ha, hb in zip(heads[0::2], heads[1::2]):
            alive = [head_gen(ha), head_gen(hb)]
            while alive:
                for g in list(alive):
                    try:
                        next(g)
                    except StopIteration:
                        alive.remove(g)
                    else:
                        yield

    def gen_wp_loads():
        wpb = pool.tile([128, 4096], BF16, name="wpb", tag="w", bufs=4)
        nc.sync.dma_start(
            wpb[:].rearrange("p (m c) -> p m c", m=4),
            wp[:, :].rearrange("(m p) c -> p m c", m=4),
        )
        for m in range(4):
            for n in range(2):
                wps[m][n] = wpb[:, 1024 * m + 512 * n:1024 * m + 512 * n + 512]
        yield

    def gen_proj(j, overlap=False):
        start_u = 0
        if overlap and j == 3:
            # open two psum groups with m=0..2 while the last attention pair
            # is still in flight; m=3 closes them once otc[3][3] exists
            t = 12
            yo = pool.tile([128, 1024], F32, name=f"yo{t}", tag="yo", bufs=2)
            pss = []
            for n in range(2):
                ps = psp.tile([128, 512], F32, name=f"yps{t}_{n}", tag="qk",
                              bufs=2)
                for m in range(3):
                    nc.tensor.matmul(
                        ps[:], otc[m][j][:, 0:128], wps[m][n],
                        start=(m == 0), stop=False,
                    )
                pss.append(ps)
                yield
            for n in range(2):
                nc.tensor.matmul(
                    pss[n][:], otc[3][j][:, 0:128], wps[3][n],
                    start=False, stop=True,
                )
                nc.scalar.copy(yo[:, 512 * n:512 * n + 512], pss[n][:])
                yield
            nc.sync.dma_start(yout[128 * t:128 * t + 128, :], yo[:])
            start_u = 1
        for u in range(start_u, 4):
            t = 4 * j + u
            yo = pool.tile([128, 1024], F32, name=f"yo{t}", tag="yo", bufs=2)
            for n in range(2):
                ps = psp.tile([128, 512], F32, name=f"yps{t}_{n}", tag="qk", bufs=2)
                for m in range(4):
                    nc.tensor.matmul(
                        ps[:], otc[m][j][:, 128 * u:128 * u + 128], wps[m][n],
                        start=(m == 0), stop=(m == 3),
                    )
                if j == 3:
                    nc.scalar.copy(yo[:, 512 * n:512 * n + 512], ps[:])
                    if u == 3:
                        nc.sync.dma_start(
                            yout[128 * t:128 * t + 128,
                                 512 * n:512 * n + 512],
                            yo[:, 512 * n:512 * n + 512])
                else:
                    nc.vector.tensor_copy(yo[:, 512 * n:512 * n + 512], ps[:])
                yield
            if not (j == 3 and u == 3):
                nc.sync.dma_start(yout[128 * t:128 * t + 128, :], yo[:])

    def chain(*gens):
        for g in gens:
            yield from g

    def merge(main, filler, ratio):
        """Pull `ratio` quanta from main, then 1 from filler, until both dry."""
        main_live = filler_live = True
        while main_live or filler_live:
            for _ in range(ratio):
                if main_live:
                    try:
                        next(main)
                    except StopIteration:
                        main_live = False
            if filler_live:
                try:
                    next(filler)
                except StopIteration:
                    filler_live = False

    def drain(g):
        for _ in g:
            pass

    drain(gen_qkv(0))
    merge(gen_attn(0), gen_qkv(1), 2)
    merge(gen_attn(1), gen_qkv(2), 3)
    merge(gen_attn(2), chain(gen_qkv(3), gen_wp_loads(), gen_proj(0)), 3)
    merge(gen_attn(3),
          chain(gen_proj(1), gen_proj(2), gen_proj(3, overlap=True)), 4)

    for m in range(4):
        qtc[m] = [None] * NJ
        otc[m] = [None] * NJ
    pool.release()
    psp.release()


def build(passes=1, dbg=False):
    key = ("nc", passes, dbg)
    if key in _CACHE:
        return _CACHE[key]
    nc = bacc.Bacc("TRN2", target_bir_lowering=False, debug=False,
                   num_devices=N_CORES)
    aps = {
        "xT": nc.dram_tensor("xT", [C, T], BF16, kind="ExternalInput").ap(),
        "wq": nc.dram_tensor("wq", [C, CH], BF16, kind="ExternalInput").ap(),
        "wk": nc.dram_tensor("wk", [C, CH], BF16, kind="ExternalInput").ap(),
        "wv": nc.dram_tensor("wv", [C, CH], BF16, kind="ExternalInput").ap(),
        "wp": nc.dram_tensor("wp", [CH, C], BF16, kind="ExternalInput").ap(),
        "bq2": nc.dram_tensor("bq2", [128, 4], F32, kind="ExternalInput").ap(),
        "bk2": nc.dram_tensor("bk2", [128, 4], F32, kind="ExternalInput").ap(),
        "mask": nc.dram_tensor("mask", [128, 128], F8, kind="ExternalInput").ap(),
        "y": nc.dram_tensor("y", [T, C], F32, kind="ExternalOutput").ap(),
    }
    dbg_aps = None
    if dbg:
        dbg_aps = {
            "qt0": nc.dram_tensor("dqt0", [128, 512], BF16,
                                  kind="ExternalOutput").ap(),
            "kt0": nc.dram_tensor("dkt0", [128, 512], BF16,
                                  kind="ExternalOutput").ap(),
            "vp0": nc.dram_tensor("dvp0", [128, 1280], F8,
                                  kind="ExternalOutput").ap(),
            "et00": nc.dram_tensor("det00", [128, 1024], F8,
                                   kind="ExternalOutput").ap(),
            "ops00": nc.dram_tensor("dops00", [65, 512], F32,
                                    kind="ExternalOutput").ap(),
            "rbs00": nc.dram_tensor("drbs00", [64, 512], F32,
                                    kind="ExternalOutput").ap(),
            "otc00": nc.dram_tensor("dotc00", [64, 512], BF16,
                                    kind="ExternalOutput").ap(),
        }
    with tile.TileContext(nc) as tc:
        for _ in range(passes):
            _emit(nc, tc, aps, dbg=dbg_aps)
    nc.compile()
    _CACHE[key] = nc
    return nc


def make_in_maps(x, Wq, bq, Wk, bk, Wv, bv, Wp, bp):
    # lower-triangle 0/1 mask (valid where s <= t) for diagonal blocks
    s_idx = np.arange(128)[:, None]
    t_idx = np.arange(128)[None, :]
    mask = (s_idx <= t_idx).astype(NP_E4)
    in_maps = []
    for c in range(N_CORES):
        b, g = c // 2, c % 2
        cols = slice(CH * g, CH * g + CH)
        in_maps.append({
            "xT": np.ascontiguousarray(x[b].T).astype(NP_BF16),
            "wq": np.ascontiguousarray(Wq[:, cols]).astype(NP_BF16),
            "wk": np.ascontiguousarray(Wk[:, cols]).astype(NP_BF16),
            "wv": np.ascontiguousarray(Wv[:, cols]).astype(NP_BF16),
            "wp": np.ascontiguousarray(Wp[cols, :] * 0.25).astype(NP_BF16),
            "bq2": np.ascontiguousarray(bq[cols].reshape(4, 128).T),
            "bk2": np.ascontiguousarray(bk[cols].reshape(4, 128).T),
            "mask": mask,
        })
    return in_maps


def kernel(x, Wq, bq, Wk, bk, Wv, bv, Wp, bp):
    # host-side prep is pure numpy; convert in case jax arrays are passed
    x, Wq, bq, Wk, bk, Wv, bv, Wp, bp = (
        np.asarray(a, dtype=np.float32)
        for a in (x, Wq, bq, Wk, bk, Wv, bv, Wp, bp)
    )
    nc = build()
    in_maps = make_in_maps(x, Wq, bq, Wk, bk, Wv, bv, Wp, bp)
    # the axon-proxied device occasionally reports a transient unrecoverable
    # exec state that clears on a fresh attempt; retry rather than fail
    last_err = None
    for _attempt in range(3):
        try:
            res = run_bass_kernel_spmd(nc, in_maps, core_ids=list(range(N_CORES)))
            break
        except Exception as e:  # noqa: BLE001
            last_err = e
            import time as _time
            _time.sleep(5)
    else:
        raise last_err
    corr = (bv @ Wp + bp).astype(np.float32)
    out = np.empty((B, T, C), dtype=np.float32)
    for b in range(B):
        out[b] = res.results[2 * b]["y"] + res.results[2 * b + 1]["y"] + corr
    return out


# revision 57
# speedup vs baseline: 1.0102x; 1.0086x over previous
"""Causal self-attention (B=4, T=2048, C=1024, H=16, D=64) on 8 trn2 cores.

Sharding: data-parallel over B (4) x tensor-parallel over head-halves (2).
Core c handles batch c//2 with heads [8*(c%2), 8*(c%2)+8). Each core emits a
partial projection output [2048, 1024]; host sums the two head-half partials
per batch and adds the (bv @ Wp + bp) correction row.

Mixed-precision layout (tuned against the rel_err<2e-2 gate; see numerics
study): the logit path (x, Wq/Wk, Q^T/K^T, S) runs in bf16 — fp8 there costs
3e-2 of error through the softmax. The attention-weight/value path runs in
fp8e4 DoubleRow: exp() writes fp8 tiles directly, and V rides as a hi/lo fp8
pair (P=e4(16v), Q=e4(16v-P), ~bf16 precision) so each AV matmul contracts
two 128-row slabs per cycle-row:
  O^T[65,t] += [P|Q]^T @ [et|et]   (rhs is the same et strip, stride-0)
Row 64 of O^T is Z (ones column in P, zeros in Q). Normalize: reciprocal of
the Z row in place, SBUF->SBUF DMA broadcast of 1/Z down 64 partitions, then
one fused (O*0.25)*(1/Z) -> otc = 4y bf16; Wp is uploaded as bf16(Wp)/4 so
the bf16 projection emits y exactly. Emission is generator-quanta: attention
head PAIRS are interleaved (two softmax chains in flight per psum ring) and
QKV/proj groups are merged in as PE filler while ACT runs the exps.
"""

import os
import sys

for _p in ("/opt/trn_rl_repo", "/root/.axon_site/_ro/trn_rl_repo"):
    if os.path.isdir(_p) and _p not in sys.path:
        sys.path.insert(0, _p)

import numpy as np
import ml_dtypes
from concourse import bacc, mybir, tile
from concourse.bass_utils import run_bass_kernel_spmd

N_CORES = 8
B, T, C = 4, 2048, 1024
H, D = 16, 64          # full model heads
HG = 8                 # heads per core (head-group)
CH = HG * D            # 512, per-core qkv width
NT = T // 128          # 16 s-tiles
NJ = T // 512          # 4 t-chunks
NC_ = C // 128         # 8 contraction tiles
F32 = mybir.dt.float32
F32R = mybir.dt.float32r
BF16 = mybir.dt.bfloat16
F8 = mybir.dt.float8e4
AF = mybir.ActivationFunctionType
DR = mybir.MatmulPerfMode.DoubleRow
MUL = mybir.AluOpType.mult
SUB = mybir.AluOpType.subtract

NP_BF16 = ml_dtypes.bfloat16
NP_E4 = ml_dtypes.float8_e4m3

_CACHE = {}


def _emit(nc, tc, aps, dbg=None):
    xT, wq, wk, wv, wp, bq2, bk2, mask, yout = (
        aps["xT"], aps["wq"], aps["wk"], aps["wv"], aps["wp"],
        aps["bq2"], aps["bk2"], aps["mask"], aps["y"],
    )

    pool = tc.alloc_tile_pool(name="pool", bufs=1)
    psp = tc.alloc_tile_pool(name="ps", bufs=1, space="PSUM")

    # ---- persistent tensors ----
    kt = [pool.tile([128, T], BF16, name=f"kt{m}", tag="kt", bufs=4) for m in range(4)]
    # V tiles: per s-tile, 8 heads x (80 P | 80 Q) fp8 (only cols 0..64 of
    # each half are used; stride 80 keeps the DoubleRow LDWEIGHTS interleave
    # step 16B-aligned). P holds e4(16v) + ones column at 64; Q holds the e4
    # residual + zero column.
    vp = [pool.tile([128, 1280], F8, name=f"vp{i}", tag="vp", bufs=NT)
          for i in range(NT)]
    tri = pool.tile([128, 128], F8, name="tri", tag="tri", bufs=1)
    bqs = pool.tile([128, 4], F32, name="bqs", tag="bias", bufs=2)
    bks = pool.tile([128, 4], F32, name="bks", tag="bias", bufs=2)
    onesf = pool.tile([128, 8], F32, name="onesf", tag="ones", bufs=2)
    zerof = pool.tile([128, 8], F32, name="zerof", tag="ones", bufs=2)
    nb1 = pool.tile([128, 1], F32, name="nb1", tag="nb", bufs=1)
    onesA = pool.tile([128, 64], F32, name="onesA", tag="onesA", bufs=2)
    ones64 = pool.tile([128, 64], F32R, name="ones64", tag="onesA", bufs=2)

    # weights as single [128, 4096] tiles (one merged DMA each; the shared
    # HWDGE device costs ~625ns per DMA instruction, so fewer+bigger wins)
    wqb = pool.tile([128, 4096], BF16, name="wqb", tag="w", bufs=4)
    wkb = pool.tile([128, 4096], BF16, name="wkb", tag="w", bufs=4)
    wvb = pool.tile([128, 4096], BF16, name="wvb", tag="w", bufs=4)
    wqs = [wqb[:, 512 * ci:512 * ci + 512] for ci in range(NC_)]
    wks = [wkb[:, 512 * ci:512 * ci + 512] for ci in range(NC_)]
    wvs = [wvb[:, 512 * ci:512 * ci + 512] for ci in range(NC_)]
    xbig = [pool.tile([128, 4096], BF16, name=f"xb{j}", tag="xt", bufs=4)
            for j in range(NJ)]

    def dma_w(eng, dst, src, lo=0, hi=8):
        eng.dma_start(
            dst[:, 512 * lo:512 * hi].rearrange("p (ci n) -> p ci n", ci=hi - lo),
            src[:, :].rearrange("(ci p) n -> p ci n", ci=8)[:, lo:hi],
        )

    def dma_x(eng, j, lo=0, hi=8):
        eng.dma_start(
            xbig[j][:, 512 * lo:512 * hi].rearrange("p (ci t) -> p ci t",
                                                    ci=hi - lo),
            xT[:, :].rearrange("(ci p) t -> p ci t", ci=8)
            [:, lo:hi, 512 * j:512 * j + 512],
        )

    # first-use tensors split across both HWDGE queues so the first QKV
    # groups can start ~2x sooner
    dma_w(nc.sync, wqb, wq, 0, 2)
    dma_x(nc.scalar, 0, 0, 2)
    dma_w(nc.sync, wqb, wq, 2, 4)
    dma_x(nc.scalar, 0, 2, 4)
    dma_w(nc.sync, wqb, wq, 4, 6)
    dma_x(nc.scalar, 0, 4, 6)
    dma_w(nc.sync, wqb, wq, 6, 8)
    dma_x(nc.scalar, 0, 6, 8)
    dma_w(nc.sync, wkb, wk, 0, 4)
    dma_w(nc.sync, wkb, wk, 4, 8)
    nc.scalar.dma_start(bqs[:], bq2[:])
    nc.scalar.dma_start(bks[:], bk2[:])
    dma_w(nc.scalar, wvb, wv)
    nc.scalar.dma_start(tri[:], mask[:])
    xt0 = [xbig[0][:, 512 * ci:512 * ci + 512] for ci in range(NC_)]
    # walrus rejects memsets with exotic value/dtype combos; stick to fp32
    # 0.0/1.0 memsets and derive everything else on DVE
    nc.gpsimd.memset(onesf[:], 1.0)
    nc.gpsimd.memset(zerof[:], 0.0)
    nc.gpsimd.memset(nb1[:], 1.0)
    nc.vector.tensor_scalar_mul(nb1[:], nb1[:], -1.0)
    nc.gpsimd.memset(onesA[:], 1.0)
    nc.vector.tensor_copy(ones64[:], onesA[:])
    for i in range(NT):
        vpr = vp[i][:, 0:1280].rearrange("p (h two e) -> p h two e", two=2, e=80)
        nc.vector.tensor_copy(vpr[:, :, 0, 64:65], onesf[:].unsqueeze(2))
        nc.vector.tensor_copy(vpr[:, :, 1, 64:65], zerof[:].unsqueeze(2))

    qtc = [[None] * NJ for _ in range(4)]   # per-chunk Q^T tiles (bf16)
    otc = [[None] * NJ for _ in range(4)]   # per-chunk otc tiles (bf16, =4y)
    wps = [[None, None] for _ in range(4)]  # wp [128,512] halves, loaded late

    def gen_qkv(j):
        """Yields after each PE group: 1 dma quantum + 12 matmul quanta."""
        if j == 0:
            xts = xt0
        else:
            dma_x(nc.sync, j)
            xts = [xbig[j][:, 512 * ci:512 * ci + 512] for ci in range(NC_)]
            yield
        for wsrc, bias_t, dst, nm in ((wqs, bqs, qtc, "qt"), (wks, bks, None, "kt")):
            for m in range(4):
                ps = psp.tile([128, 512], F32, name=f"{nm}ps{j}_{m}", tag="qk", bufs=2)
                for ci in range(NC_):
                    nc.tensor.matmul(
                        ps[:], wsrc[ci][:, 128 * m:128 * m + 128], xts[ci][:],
                        start=(ci == 0), stop=(ci == NC_ - 1),
                    )
                if dst is None:
                    out_ap = kt[m][:, 512 * j:512 * j + 512]
                else:
                    t_ = pool.tile([128, 512], BF16, name=f"qt{m}_{j}", tag="qtc",
                                   bufs=8)
                    dst[m][j] = t_
                    out_ap = t_[:]
                nc.vector.tensor_scalar_add(out_ap, ps[:], bias_t[:, m:m + 1])
                if dbg is not None and j == 0 and m == 0:
                    if dst is None:
                        nc.scalar.dma_start(dbg["kt0"][:, :],
                                            kt[0][:, 0:512])
                    else:
                        nc.scalar.dma_start(dbg["qt0"][:, :], t_[:])
                yield
        for u in range(4):
            i = 4 * j + u
            ps = psp.tile([128, 512], F32, name=f"vps{i}", tag="qk", bufs=2)
            for ci in range(NC_):
                nc.tensor.matmul(
                    ps[:], xts[ci][:, 128 * u:128 * u + 128], wvs[ci][:],
                    start=(ci == 0), stop=(ci == NC_ - 1),
                )
            vpr = vp[i][:, 0:1280].rearrange("p (h two e) -> p h two e",
                                             two=2, e=80)
            src = ps[:].rearrange("p (h e) -> p h e", e=64)
            # P = e4(16 v); Q = e4(16 v - P)
            nc.vector.tensor_scalar_mul(vpr[:, :, 0, 0:64], src, 16.0)
            nc.vector.scalar_tensor_tensor(
                vpr[:, :, 1, 0:64], in0=src, scalar=16.0,
                in1=vpr[:, :, 0, 0:64], op0=MUL, op1=SUB,
            )
            if dbg is not None and i == 0:
                nc.scalar.dma_start(dbg["vp0"][:, :], vp[0][:])
            yield

    def gen_attn(j, heads=(1, 0, 3, 2, 5, 4, 7, 6)):
        n_i = 4 * j + 4

        def tile_layout(p):
            # pairs of s-tiles per [128,1024] PSUM slot; diagonal tiles are
            # narrowed to the causally valid t-range [128r, 512).
            # entries: (i, slot_col, valid_t0, width, diag_block_col)
            i0, i1 = 2 * p, 2 * p + 1
            r0_, r1_ = i0 - 4 * j, i1 - 4 * j
            if r1_ < 0:
                return [(i0, 0, 0, 512, None), (i1, 512, 0, 512, None)], 1024
            if r0_ == 0:
                return [(i0, 0, 0, 512, 0), (i1, 512, 128, 384, 512)], 896
            return [(i0, 0, 256, 256, 0), (i1, 256, 384, 128, 256)], 384

        # odd heads first: their normalize chain ends in a partition-shifting
        # SBUF->SBUF DMA, so keep an even (cheap-chain) head last
        def head_gen(h):
            mt = h // 2
            off = 64 * (h % 2)
            ops = psp.tile([65, 512], F32, name=f"ops{h}_{j}", tag="o", bufs=2)
            qsrc = qtc[mt][j][off:off + 64, :]
            first_av = True

            def emit_av(layout, et, p):
                nonlocal first_av
                for (i, scol, t0, w, dcol) in layout:
                    if dcol is not None:
                        blk = et[:, dcol:dcol + 128]
                        nc.gpsimd.tensor_mul(blk, blk, tri[:])
                    vp_lhsT = vp[i][:, 160 * h:160 * h + 160].rearrange(
                        "p (two e) -> p two e", two=2)[:, :, 0:65]
                    for c0 in range(0, w, 256):
                        wc = min(256, w - c0)
                        rhs = et[:, scol + c0:scol + c0 + wc].unsqueeze(1) \
                            .broadcast_to([128, 2, wc])
                        nc.tensor.matmul(
                            ops[:, t0 + c0:t0 + c0 + wc], vp_lhsT, rhs,
                            start=first_av, stop=(i == n_i - 1 and c0 + wc == w),
                            perf_mode=DR,
                        )
                        first_av = False
                if dbg is not None and h == 0 and j == 0 and p == 0:
                    nc.scalar.dma_start(dbg["et00"][:, :], et[:])

            pending = None   # software pipeline: AV(p) emitted after S(p+1)
            for p in range(n_i // 2):
                layout, exp_hi = tile_layout(p)
                sp = psp.tile([128, 1024], F32, name=f"sp{h}_{j}_{p}", tag="sp",
                              bufs=2)
                for (i, scol, t0, w, _) in layout:
                    nc.tensor.matmul(
                        sp[:, scol:scol + w],
                        kt[mt][off:off + 64, 128 * i:128 * i + 128],
                        qsrc[:, t0:t0 + w],
                        start=True, stop=True,
                    )
                et = pool.tile([128, 1024], F8, name=f"et{h}_{j}_{p}", tag="et",
                               bufs=6)
                nc.scalar.activation(et[:, 0:exp_hi], sp[:, 0:exp_hi], AF.Exp,
                                     scale=0.125, bias=nb1[:])
                if pending is not None:
                    emit_av(*pending)
                pending = (layout, et, p)
                yield
            emit_av(*pending)
            # normalize: rows 0..63 unnormalized O^T (=16 y Z), row 64 = Z
            # 1/Z in place at partition 64 (fp32r), PE-broadcast to the 64
            # O^T partitions, then one fused (O * 0.25) * (1/Z) -> otc = 4y
            zr = pool.tile([65, 512], F32R, name=f"zr{h}_{j}", tag="zr", bufs=2)
            with nc.allow_low_precision(reason="fp32r rounding of softmax denom"):
                nc.vector.reciprocal(zr[64:65, :], ops[64:65, :])
            rbs = pool.tile([64, 512], F32R, name=f"rbs{h}_{j}", tag="rbs", bufs=2)
            if j == 3 and h in (6, 7):
                # tail: PE is idle and the DMA round-trip would sit on the
                # critical path into proj3 -- broadcast via PE instead
                rbp = psp.tile([64, 512], F32, name=f"rbp{h}", tag="qk", bufs=2)
                nc.tensor.matmul(rbp[:], ones64[64:65, :], zr[64:65, :],
                                 start=True, stop=True)
                nc.vector.tensor_copy(rbs[:], rbp[:])
            else:
                with nc.allow_non_contiguous_dma(reason="1/Z partition broadcast"):
                    nc.sync.dma_start(
                        rbs[:],
                        zr[64:65, :].unsqueeze(1).broadcast_to([1, 64, 512]))
            if dbg is not None and h == 0 and j == 0:
                opc = pool.tile([65, 512], F32, name="dbgopc", tag="dbgo", bufs=1)
                nc.vector.tensor_copy(opc[:], ops[:])
                nc.scalar.dma_start(dbg["ops00"][:, :], opc[:])
                nc.scalar.dma_start(dbg["rbs00"][:, :], rbs[:].bitcast(F32))
            if otc[mt][j] is None:
                otc[mt][j] = pool.tile([128, 512], BF16, name=f"ot{mt}_{j}",
                                       tag="otc", bufs=16)
            if h % 2 == 0:
                nc.vector.scalar_tensor_tensor(
                    otc[mt][j][0:64, :], in0=ops[0:64, :], scalar=0.25,
                    in1=rbs[:], op0=MUL, op1=MUL)
                if dbg is not None and h == 0 and j == 0:
                    nc.scalar.dma_start(dbg["otc00"][:, :], otc[0][0][0:64, :])
            else:
                st = pool.tile([64, 512], BF16, name=f"st{h}_{j}", tag="st", bufs=1)
                nc.vector.scalar_tensor_tensor(
                    st[:], in0=ops[0:64, :], scalar=0.25,
                    in1=rbs[:], op0=MUL, op1=MUL)
                nc.sync.dma_start(otc[mt][j][64:128, :], st[:])
            yield

        for ha, hb in zip(heads[0::2], heads[1::2]):
            alive = [head_gen(ha), head_gen(hb)]
            while alive:
                for g in list(alive):
                    try:
                        next(g)
                    except StopIteration:
                        alive.remove(g)
                    else:
                        yield

    def gen_wp_loads():
        wpb = pool.tile([128, 4096], BF16, name="wpb", tag="w", bufs=4)
        nc.sync.dma_start(
            wpb[:].rearrange("p (m c) -> p m c", m=4),
            wp[:, :].rearrange("(m p) c -> p m c", m=4),
        )
        for m in range(4):
            for n in range(2):
                wps[m][n] = wpb[:, 1024 * m + 512 * n:1024 * m + 512 * n + 512]
        yield

    def gen_proj(j, overlap=False):
        start_u = 0
        if overlap and j == 3:
            # open two psum groups with m=0..2 while the last attention pair
            # is still in flight; m=3 closes them once otc[3][3] exists
            t = 12
            yo = pool.tile([128, 1024], F32, name=f"yo{t}", tag="yo", bufs=2)
            pss = []
            for n in range(2):
                ps = psp.tile([128, 512], F32, name=f"yps{t}_{n}", tag="qk",
                              bufs=2)
                for m in range(3):
                    nc.tensor.matmul(
                        ps[:], otc[m][j][:, 0:128], wps[m][n],
                        start=(m == 0), stop=False,
                    )
                pss.append(ps)
                yield
            for n in range(2):
                nc.tensor.matmul(
                    pss[n][:], otc[3][j][:, 0:128], wps[3][n],
                    start=False, stop=True,
                )
                nc.scalar.copy(yo[:, 512 * n:512 * n + 512], pss[n][:])
                yield
            nc.sync.dma_start(yout[128 * t:128 * t + 128, :], yo[:])
            start_u = 1
        for u in range(start_u, 4):
            t = 4 * j + u
            yo = pool.tile([128, 1024], F32, name=f"yo{t}", tag="yo", bufs=2)
            for n in range(2):
                ps = psp.tile([128, 512], F32, name=f"yps{t}_{n}", tag="qk", bufs=2)
                for m in range(4):
                    nc.tensor.matmul(
                        ps[:], otc[m][j][:, 128 * u:128 * u + 128], wps[m][n],
                        start=(m == 0), stop=(m == 3),
                    )
                if j == 3:
                    nc.scalar.copy(yo[:, 512 * n:512 * n + 512], ps[:])
                    if u == 3:
                        nc.sync.dma_start(
                            yout[128 * t:128 * t + 128,
                                 512 * n:512 * n + 512],
                            yo[:, 512 * n:512 * n + 512])
                else:
                    nc.vector.tensor_copy(yo[:, 512 * n:512 * n + 512], ps[:])
                yield
            if not (j == 3 and u == 3):
                nc.sync.dma_start(yout[128 * t:128 * t + 128, :], yo[:])

    def chain(*gens):
        for g in gens:
            yield from g

    def merge(main, filler, ratio):
        """Pull `ratio` quanta from main, then 1 from filler, until both dry."""
        main_live = filler_live = True
        while main_live or filler_live:
            for _ in range(ratio):
                if main_live:
                    try:
                        next(main)
                    except StopIteration:
                        main_live = False
            if filler_live:
                try:
                    next(filler)
                except StopIteration:
                    filler_live = False

    def drain(g):
        for _ in g:
            pass

    drain(gen_qkv(0))
    merge(gen_attn(0), gen_qkv(1), 2)
    merge(gen_attn(1), gen_qkv(2), 2)
    merge(gen_attn(2), chain(gen_qkv(3), gen_wp_loads(), gen_proj(0)), 4)
    merge(gen_attn(3),
          chain(gen_proj(1), gen_proj(2), gen_proj(3, overlap=True)), 4)

    for m in range(4):
        qtc[m] = [None] * NJ
        otc[m] = [None] * NJ
    pool.release()
    psp.release()


def build(passes=1, dbg=False):
    key = ("nc", passes, dbg)
    if key in _CACHE:
        return _CACHE[key]
    nc = bacc.Bacc("TRN2", target_bir_lowering=False, debug=False,
                   num_devices=N_CORES)
    aps = {
        "xT": nc.dram_tensor("xT", [C, T], BF16, kind="ExternalInput").ap(),
        "wq": nc.dram_tensor("wq", [C, CH], BF16, kind="ExternalInput").ap(),
        "wk": nc.dram_tensor("wk", [C, CH], BF16, kind="ExternalInput").ap(),
        "wv": nc.dram_tensor("wv", [C, CH], BF16, kind="ExternalInput").ap(),
        "wp": nc.dram_tensor("wp", [CH, C], BF16, kind="ExternalInput").ap(),
        "bq2": nc.dram_tensor("bq2", [128, 4], F32, kind="ExternalInput").ap(),
        "bk2": nc.dram_tensor("bk2", [128, 4], F32, kind="ExternalInput").ap(),
        "mask": nc.dram_tensor("mask", [128, 128], F8, kind="ExternalInput").ap(),
        "y": nc.dram_tensor("y", [T, C], F32, kind="ExternalOutput").ap(),
    }
    dbg_aps = None
    if dbg:
        dbg_aps = {
            "qt0": nc.dram_tensor("dqt0", [128, 512], BF16,
                                  kind="ExternalOutput").ap(),
            "kt0": nc.dram_tensor("dkt0", [128, 512], BF16,
                                  kind="ExternalOutput").ap(),
            "vp0": nc.dram_tensor("dvp0", [128, 1280], F8,
                                  kind="ExternalOutput").ap(),
            "et00": nc.dram_tensor("det00", [128, 1024], F8,
                                   kind="ExternalOutput").ap(),
            "ops00": nc.dram_tensor("dops00", [65, 512], F32,
                                    kind="ExternalOutput").ap(),
            "rbs00": nc.dram_tensor("drbs00", [64, 512], F32,
                                    kind="ExternalOutput").ap(),
            "otc00": nc.dram_tensor("dotc00", [64, 512], BF16,
                                    kind="ExternalOutput").ap(),
        }
    with tile.TileContext(nc) as tc:
        for _ in range(passes):
            _emit(nc, tc, aps, dbg=dbg_aps)
    nc.compile()
    _CACHE[key] = nc
    return nc


def make_in_maps(x, Wq, bq, Wk, bk, Wv, bv, Wp, bp):
    # lower-triangle 0/1 mask (valid where s <= t) for diagonal blocks
    s_idx = np.arange(128)[:, None]
    t_idx = np.arange(128)[None, :]
    mask = (s_idx <= t_idx).astype(NP_E4)
    in_maps = []
    for c in range(N_CORES):
        b, g = c // 2, c % 2
        cols = slice(CH * g, CH * g + CH)
        in_maps.append({
            "xT": np.ascontiguousarray(x[b].T).astype(NP_BF16),
            "wq": np.ascontiguousarray(Wq[:, cols]).astype(NP_BF16),
            "wk": np.ascontiguousarray(Wk[:, cols]).astype(NP_BF16),
            "wv": np.ascontiguousarray(Wv[:, cols]).astype(NP_BF16),
            "wp": np.ascontiguousarray(Wp[cols, :] * 0.25).astype(NP_BF16),
            "bq2": np.ascontiguousarray(bq[cols].reshape(4, 128).T),
            "bk2": np.ascontiguousarray(bk[cols].reshape(4, 128).T),
            "mask": mask,
        })
    return in_maps


def kernel(x, Wq, bq, Wk, bk, Wv, bv, Wp, bp):
    # host-side prep is pure numpy; convert in case jax arrays are passed
    x, Wq, bq, Wk, bk, Wv, bv, Wp, bp = (
        np.asarray(a, dtype=np.float32)
        for a in (x, Wq, bq, Wk, bk, Wv, bv, Wp, bp)
    )
    nc = build()
    in_maps = make_in_maps(x, Wq, bq, Wk, bk, Wv, bv, Wp, bp)
    # the axon-proxied device occasionally reports a transient unrecoverable
    # exec state that clears on a fresh attempt; retry rather than fail
    last_err = None
    for _attempt in range(3):
        try:
            res = run_bass_kernel_spmd(nc, in_maps, core_ids=list(range(N_CORES)))
            break
        except Exception as e:  # noqa: BLE001
            last_err = e
            import time as _time
            _time.sleep(5)
    else:
        raise last_err
    corr = (bv @ Wp + bp).astype(np.float32)
    out = np.empty((B, T, C), dtype=np.float32)
    for b in range(B):
        out[b] = res.results[2 * b]["y"] + res.results[2 * b + 1]["y"] + corr
    return out
